# revision 1
# baseline (speedup 1.0000x reference)
"""Trainium2 Bass kernel for nn_BCTransformer: B=131072 batch of tiny 2-token
4-layer transformer encoder forward passes.

Strategy: pure data parallel over 8 NeuronCores (16384 batch each).  Within a
core, activations live feature-major [D=128 partitions, columns], columns =
(token, batch).  The whole network is fused in SBUF per super-tile of 1024
columns (512 batch x 2 tokens); 32 super-tiles per core.

Key tricks:
 - matmuls run in float32r (full PE rate, ~1e-4 rel err) via AP bitcast.
 - LayerNorm over the partition dim: centering matrix (I - J/128) as one
   matmul, variance via (J/256) matmul on Square(hc), rstd via a custom DVE
   op (bit-trick seed + 2 Newton iterations).  No ACT table needed.
 - softmax over S=2 tokens == sigmoid == 0.5 + 0.5*tanh(d/2); attention out
   o_q = (v0+v1)/2 + 0.5*t_q*(v0-v1), with the 0.5s folded into W_out.
 - Exact gelu / tanh / square all live in one ACT table set (gelu_and_others)
   => zero table switches.
 - Linear biases + LN affine folds ride psum->sbuf copies or are folded into
   the next layer's weights host-side.  Residual adds are identity matmuls
   accumulated into PSUM.
"""
import sys

sys.path.insert(0, "/opt/trn_rl_repo")

import math
from contextlib import ExitStack

import numpy as np

import concourse.bass as bass
import concourse.tile as tile
from concourse import bacc, mybir
from concourse.bass_utils import run_bass_kernel_spmd

# ---------------------------------------------------------------- constants
D = 128
NH = 4
HD = 32
FF = 256
L = 4
S = 2
B = 131072
EPS = 1e-5
NCORES = 8
BP = B // NCORES          # batch per core = 16384
N = 512                   # batch elems per super-tile
NT = 2 * N                # columns per super-tile (tok0 block | tok1 block)
NTILES = BP // N          # 32
MMC = 512                 # matmul column chunk

F32 = mybir.dt.float32
F32R = mybir.dt.float32r
I32 = mybir.dt.int32
AF = mybir.ActivationFunctionType
ALU = mybir.AluOpType

# ------------------------------------------------- custom DVE rsqrt op
MAGIC = 0x5F375A86
MAGIC_VH = MAGIC - (1 << 22)   # seed computed from bits of vh = v/2
SEED_ADD = MAGIC_VH + 1        # seed = ~(i_vh >> 1) + SEED_ADD


def _register_rsqrt_op():
    import concourse.dve_ops as dve_ops
    from concourse.dve_ops import DveOp
    from concourse.dve_spec import C0, Spec, Src0, Src1, lower, _has_src1
    from concourse.dve_uop import DveOpSpec

    name = "RSQRT_NR2_ANT"
    if name in dve_ops._SUB_OPCODE_FOR_NAME:
        for op in dve_ops.OPS:
            if op.name == name:
                return op

    def _ref(in0, in1, c0, c1, c2):
        vh = in0.astype(np.float32)
        s = in1.astype(np.float32)
        y1 = s * (c0 - vh * s * s)
        y2 = y1 * (c0 - vh * y1 * y1)
        return y2.astype(np.float32)

    _y1 = Src1 * (C0 - Src0 * (Src1 * Src1))
    spec = Spec(body=_y1 * (C0 - Src0 * (_y1 * _y1)), reference=_ref)
    opcode = dve_ops._CUSTOM_DVE_ROW_BASE + len(dve_ops.OPS)
    assert opcode < 0x20
    dve_ops._SUB_OPCODE_FOR_NAME[name] = opcode
    shas = {}
    for ver in ("v3", "v4"):
        try:
            uops = lower(spec, ver=ver)
            shas[ver] = DveOpSpec(
                name=name, opcode=opcode, uops=uops, rd1_en=_has_src1(spec)
            ).sha(ver)
        except Exception:
            pass
    op = DveOp(name, spec, subdim=False, uops_sha=shas)
    dve_ops.OPS.append(op)
    dve_ops.CUSTOM_DVE_SPECS[name] = spec
    return op


RSQRT_NR2 = _register_rsqrt_op()


# ------------------------------------------------- host-side weight folding
def _prepare_weights(p):
    f = lambda a: np.asarray(a, np.float32)
    out = {}
    out["w_inT"] = np.ascontiguousarray((f(p["w_in"]) * math.sqrt(D)).T)
    b_in = (f(p["b_in"]) * math.sqrt(D))[:, None]
    pos = np.arange(10, dtype=np.float32)[:, None]
    div = np.exp(np.arange(0, D, 2, dtype=np.float32) * (-math.log(10000.0) / D))
    pe = np.zeros((10, D), dtype=np.float32)
    pe[:, 0::2] = np.sin(pos * div)
    pe[:, 1::2] = np.cos(pos * div)
    g_in = f(p["g_in"])[:, None]
    bias_e0 = (f(p["bt_in"]) + pe[0])[:, None]
    bias_e1 = (f(p["bt_in"]) + pe[1])[:, None]
    out["eb"] = np.ascontiguousarray(
        np.concatenate([b_in, g_in, bias_e0, bias_e1], axis=1))  # [128,4]

    wl = np.zeros((L, 128, 1024), np.float32)
    blb = np.zeros((L, 128, 8), np.float32)
    for l in range(L):
        g1 = f(p["n1_g"][l]); b1 = f(p["n1_b"][l])
        qkv_w = f(p["qkv_w"][l]); qkv_b = f(p["qkv_b"][l])
        wqkvT = (qkv_w * g1[None, :]).T          # [128, 384]
        bqkv = qkv_b + qkv_w @ b1
        out_w = f(p["out_w"][l])
        woT_half = (0.5 * out_w).T               # [128,128]
        g2 = f(p["n2_g"][l]); b2 = f(p["n2_b"][l])
        ff1_w = f(p["ff1_w"][l]); ff1_b = f(p["ff1_b"][l])
        ff1T = (ff1_w * g2[None, :]).T           # [128, 256]
        bff1 = ff1_b + ff1_w @ b2
        ff2T = f(p["ff2_w"][l]).T                # [256, 128]
        wl[l, :, 0:384] = wqkvT
        wl[l, :, 384:512] = woT_half
        wl[l, :, 512:768] = ff1T
        wl[l, :, 768:896] = ff2T[0:128]
        wl[l, :, 896:1024] = ff2T[128:256]
        blb[l, :, 0] = bqkv[0:128]
        blb[l, :, 1] = bqkv[128:256]
        blb[l, :, 2] = bqkv[256:384]
        blb[l, :, 3] = f(p["out_b"][l])
        blb[l, :, 4] = bff1[0:128]
        blb[l, :, 5] = bff1[128:256]
        blb[l, :, 6] = f(p["ff2_b"][l])
    out["wl"] = wl
    out["bl"] = blb

    go = f(p["g_out"]); bo = f(p["bt_out"])
    h1_w = f(p["h1_w"])
    wh = np.zeros((128, 193), np.float32)
    wh[:, 0:128] = (0.5 * h1_w * go[None, :]).T
    wh[:, 128:192] = f(p["h2_w"]).T
    wh[0:64, 192] = f(p["h3_w"])[0]
    out["wh"] = wh
    bh = np.zeros((128, 3), np.float32)
    bh[:, 0] = f(p["h1_b"]) + h1_w @ bo
    bh[0:64, 1] = f(p["h2_b"])
    bh[0, 2] = f(p["h3_b"])[0]
    out["bh"] = bh
    return out


def _static_consts():
    c = {}
    c["C"] = (np.eye(128, dtype=np.float32) - 1.0 / 128.0).astype(np.float32)
    c["Jv"] = np.full((128, 128), 1.0 / 256.0, np.float32)
    c["I"] = np.eye(128, dtype=np.float32)
    c["epsrow"] = np.full((1, 128), EPS / 2.0, np.float32)
    sm = np.zeros((128, 4), np.float32)
    for d in range(128):
        sm[d, d // HD] = 1.0 / math.sqrt(HD)
    c["smask"] = sm
    bc = np.zeros((36, 256), np.float32)
    for d in range(128):
        bc[0 + d // HD, 0 * 128 + d] = 1.0
        bc[32 + d // HD, 1 * 128 + d] = 1.0
    c["bcmask"] = bc
    return c


def r32(ap):
    return ap.bitcast(F32R)


def _mm(nc, out_ps, lhsT, rhs, start, stop):
    """float32r matmul, chunked over the free dim (<=MMC cols per call)."""
    n = rhs.shape[-1]
    nch = (n + MMC - 1) // MMC
    for c in range(nch):
        sl = slice(c * MMC, min((c + 1) * MMC, n))
        nc.tensor.matmul(out_ps[:, sl], lhsT, rhs[:, sl],
                         start=start, stop=stop)


def build_nc(ntiles=NTILES):
    nc = bacc.Bacc(None, target_bir_lowering=False)
    cst = _static_consts()

    x_d = nc.dram_tensor("x", [BP, 4], F32, kind="ExternalInput")
    wl_d = nc.dram_tensor("wl", [L, 128, 1024], F32, kind="ExternalInput")
    bl_d = nc.dram_tensor("bl", [L, 128, 8], F32, kind="ExternalInput")
    winT_d = nc.dram_tensor("w_inT", [2, 128], F32, kind="ExternalInput")
    eb_d = nc.dram_tensor("eb", [128, 4], F32, kind="ExternalInput")
    wh_d = nc.dram_tensor("wh", [128, 193], F32, kind="ExternalInput")
    bh_d = nc.dram_tensor("bh", [128, 3], F32, kind="ExternalInput")
    o_d = nc.dram_tensor("o", [1, BP], F32, kind="ExternalOutput")

    C_d = nc.inline_tensor(cst["C"], name="Cmat")
    Jv_d = nc.inline_tensor(cst["Jv"], name="Jvmat")
    I_d = nc.inline_tensor(cst["I"], name="Imat")
    eps_d = nc.inline_tensor(cst["epsrow"], name="epsrow")
    sm_d = nc.inline_tensor(cst["smask"], name="smask")
    bc_d = nc.inline_tensor(cst["bcmask"], name="bcmask")

    with tile.TileContext(nc) as tc, ExitStack() as ctx:
        wp = ctx.enter_context(tc.tile_pool(name="weights", bufs=1))
        hp = ctx.enter_context(tc.tile_pool(name="hbuf", bufs=3))
        sp = ctx.enter_context(tc.tile_pool(name="scratch", bufs=2))
        qp = ctx.enter_context(tc.tile_pool(name="qkv", bufs=3))
        psA = ctx.enter_context(tc.tile_pool(name="psA", bufs=2, space="PSUM"))
        psB = ctx.enter_context(tc.tile_pool(name="psB", bufs=4, space="PSUM"))

        def wtile(src, shape, tag):
            t = wp.tile(shape, F32, tag=tag)
            nc.sync.dma_start(t[:], src)
            return t

        def wtile_r(src, shape, tag):
            st = sp.tile([128, 1024], F32, tag="wstage")
            sv = st[: shape[0], : shape[1]]
            nc.sync.dma_start(sv, src)
            t = wp.tile(shape, F32R, tag=tag)
            nc.scalar.copy(t[:], sv)
            return t

        wl_t = [wtile_r(wl_d[l], [128, 1024], f"wl{l}") for l in range(L)]
        bl_t = [wtile(bl_d[l], [128, 8], f"bl{l}") for l in range(L)]
        winT_t = wtile_r(winT_d[:], [2, 128], "winT")
        eb_t = wtile(eb_d[:], [128, 4], "eb")
        wh_t = wtile_r(wh_d[:], [128, 193], "wh")
        bh_t = wtile(bh_d[:], [128, 3], "bh")
        C_t = wtile_r(C_d[:], [128, 128], "Cm")
        Jv_t = wtile_r(Jv_d[:], [128, 128], "Jv")
        I_t = wtile_r(I_d[:], [128, 128], "Im")
        eps_t = wtile_r(eps_d[:], [1, 128], "epsr")
        sm_t = wtile_r(sm_d[:], [128, 4], "smask")
        sm_bb = wp.tile([128, 4], mybir.dt.bfloat16, tag="smaskb")
        nc.vector.tensor_copy(sm_bb[:], sm_t[:].bitcast(F32))
        bc_t = wtile_r(bc_d[:], [36, 256], "bcm")
        ones_s = sp.tile([128, 1024], F32, tag="wstage")
        nc.vector.memset(ones_s[0:1, 0:NT], 1.0)
        ones_t = wp.tile([1, NT], F32R, tag="ones")
        nc.scalar.copy(ones_t[:], ones_s[0:1, 0:NT])

        b_in_ap = eb_t[:, 0:1]
        g_in_ap = eb_t[:, 1:2]
        bias_e0 = eb_t[:, 2:3]
        bias_e1 = eb_t[:, 3:4]

        def layernorm(hsb, tag="", y_dt=F32R):
            """y = (h - mean)/sqrt(var + eps), [128, NT] sbuf tile.
            Pipelined in two column halves (1-bank psum tiles) so the
            serial square->var->seed->rsqrt->apply chain overlaps."""
            y = sp.tile([128, NT], y_dt, tag="yln")
            for hf in range(2):
                cs = slice(hf * N, (hf + 1) * N)
                hc_ps = psB.tile([128, N], F32, tag="lnh")
                nc.tensor.matmul(hc_ps[:], C_t[:], hsb[:, cs],
                                 start=True, stop=True)
                sq = sp.tile([128, N], F32R, tag="sq")
                nc.scalar.activation(out=sq[:], in_=hc_ps[:], func=AF.Square,
                                     bias=0.0, scale=1.0)
                vh_ps = psB.tile([128, N], F32, tag="lnh")
                nc.tensor.matmul(vh_ps[:], eps_t[:], ones_t[:, 0:N],
                                 start=True, stop=False)
                nc.tensor.matmul(vh_ps[:], Jv_t[:], sq[:],
                                 start=False, stop=True)
                tb = sp.tile([128, N], I32, tag="tbits")
                nc.vector.tensor_scalar(
                    tb[:], vh_ps[:].bitcast(I32), 1, 0xFFFFFFFF,
                    op0=ALU.logical_shift_right, op1=ALU.bitwise_xor)
                nc.gpsimd.tensor_scalar(tb[:], tb[:], SEED_ADD, None,
                                         op0=ALU.add)
                R = sp.tile([128, N], F32, tag="rstd")
                nc.vector._custom_dve(RSQRT_NR2, out=R[:], in0=vh_ps[:],
                                      in1=tb[:].bitcast(F32), s0=1.5)
                nc.vector.tensor_mul(y[:, cs], hc_ps[:], R[:])
            return y

        def emit_embed(it):
            b0 = it * N
            xs0 = sp.tile([2, N], F32, tag="xs0")
            xs1 = sp.tile([2, N], F32, tag="xs1")
            xs = x_d[b0:b0 + N, :]
            nc.sync.dma_start(xs0[:], xs.rearrange("n f -> f n")[0:2, :])
            nc.sync.dma_start(xs1[:], xs.rearrange("n f -> f n")[2:4, :])
            xt0 = sp.tile([2, N], F32R, tag="xt0")
            nc.scalar.copy(xt0[:], xs0[:])
            xt1 = sp.tile([2, N], F32R, tag="xt1")
            nc.scalar.copy(xt1[:], xs1[:])

            pe0 = psA.tile([128, NT], F32, tag="psbig")
            _mm(nc, pe0[:, 0:N], winT_t[:], xt0[:], start=True, stop=True)
            _mm(nc, pe0[:, N:NT], winT_t[:], xt1[:], start=True, stop=True)
            h_emb = sp.tile([128, NT], F32R, tag="hemb")
            nc.scalar.activation(out=h_emb[:], in_=pe0[:], func=AF.Identity,
                                 bias=b_in_ap, scale=1.0)

            y_e = layernorm(h_emb, tag="e", y_dt=F32)
            h = hp.tile([128, NT], F32R, tag="h")
            nc.vector.tensor_scalar(h[:, 0:N], y_e[:, 0:N], g_in_ap, bias_e0,
                                    op0=ALU.mult, op1=ALU.add)
            nc.vector.tensor_scalar(h[:, N:NT], y_e[:, N:NT], g_in_ap, bias_e1,
                                    op0=ALU.mult, op1=ALU.add)
            return h

        def emit_layer(l, h):
            W = wl_t[l]
            Bb = bl_t[l]
            y1 = layernorm(h, tag="1")
            qkv_sb = []
            for j in range(3):
                ps = psA.tile([128, NT], F32, tag="psbig")
                _mm(nc, ps, W[:, 128 * j:128 * (j + 1)], y1[:],
                    start=True, stop=True)
                dt_j = mybir.dt.bfloat16 if j < 2 else F32R
                t = qp.tile([128, NT], dt_j, tag=f"qkv{j}")
                if j == 0:
                    nc.vector.tensor_scalar(t[:], ps[:], Bb[:, j:j + 1], None,
                                            op0=ALU.add)
                else:
                    nc.scalar.activation(out=t[:], in_=ps[:], func=AF.Identity,
                                         bias=Bb[:, j:j + 1], scale=1.0)
                qkv_sb.append(t)
            q_sb, k_sb, v_sb = qkv_sb
            dk = sp.tile([128, N], mybir.dt.bfloat16, tag="dk")
            nc.gpsimd.tensor_tensor(dk[:], k_sb[:, 0:N], k_sb[:, N:NT],
                                    op=ALU.subtract)
            pr = sp.tile([128, 2, N], mybir.dt.bfloat16, tag="prods")
            apk = dk[:]
            dk_b = bass.AP(tensor=apk.tensor, offset=apk.offset,
                           ap=[apk.ap[0], [0, 2], apk.ap[1]])
            nc.vector.tensor_mul(
                pr[:], q_sb[:].rearrange("p (q n) -> p q n", q=2), dk_b)
            d_ps = psB.tile([36, N], F32, tag="lnh")
            nc.tensor.matmul(d_ps[0:4, :], sm_bb[:], pr[:, 0, :],
                             start=True, stop=True)
            nc.tensor.matmul(d_ps[32:36, :], sm_bb[:], pr[:, 1, :],
                             start=True, stop=True, tile_position=(0, 32))
            T8 = sp.tile([36, N], F32R, tag="T8")
            nc.scalar.activation(out=T8[:], in_=d_ps[:],
                                 func=AF.Tanh, bias=0.0, scale=0.5)
            dv = sp.tile([128, N], mybir.dt.bfloat16, tag="dv")
            vf = v_sb[:].bitcast(F32)
            nc.gpsimd.tensor_tensor(dv[:], vf[:, 0:N], vf[:, N:NT],
                                    op=ALU.subtract)
            tb_ps = psA.tile([128, NT], F32, tag="psbig")
            nc.tensor.matmul(tb_ps[:, 0:N], bc_t[:, 0:128], T8[:],
                             start=True, stop=True)
            nc.tensor.matmul(tb_ps[:, N:NT], bc_t[:, 128:256], T8[:],
                             start=True, stop=True)
            u = sp.tile([128, NT], F32R, tag="u")
            ap0 = dv[:]
            dv_b = bass.AP(tensor=ap0.tensor, offset=ap0.offset,
                           ap=[ap0.ap[0], [0, 2], ap0.ap[1]])
            nc.vector.tensor_mul(
                u[:].rearrange("p (q n) -> p q n", q=2),
                tb_ps[:].rearrange("p (q n) -> p q n", q=2), dv_b)
            p1 = psA.tile([128, NT], F32, tag="psbig")
            woT = W[:, 384:512]
            _mm(nc, p1, I_t[:], h[:], start=True, stop=False)
            for qi in range(2):
                sl = slice(qi * N, (qi + 1) * N)
                nc.tensor.matmul(p1[:, sl], woT, v_sb[:, 0:N],
                                 start=False, stop=False)
                nc.tensor.matmul(p1[:, sl], woT, v_sb[:, N:NT],
                                 start=False, stop=False)
                nc.tensor.matmul(p1[:, sl], woT, u[:, sl],
                                 start=False, stop=True)
            h2t = hp.tile([128, NT], F32R, tag="h")
            nc.scalar.activation(out=h2t[:], in_=p1[:], func=AF.Identity,
                                 bias=Bb[:, 3:4], scale=1.0)
            h = h2t
            y2 = layernorm(h, tag="2")
            f0 = psA.tile([128, NT], F32, tag="psbig")
            _mm(nc, f0, W[:, 512:640], y2[:], start=True, stop=True)
            f1 = psA.tile([128, NT], F32, tag="psbig")
            _mm(nc, f1, W[:, 640:768], y2[:], start=True, stop=True)
            g0 = sp.tile([128, NT], F32R, tag="g0")
            nc.scalar.activation(out=g0[:], in_=f0[:], func=AF.Gelu,
                                 bias=Bb[:, 4:5], scale=1.0)
            g1 = sp.tile([128, NT], F32R, tag="g1")
            nc.scalar.activation(out=g1[:], in_=f1[:], func=AF.Gelu,
                                 bias=Bb[:, 5:6], scale=1.0)
            p2 = psA.tile([128, NT], F32, tag="psbig")
            _mm(nc, p2, I_t[:], h[:], start=True, stop=False)
            _mm(nc, p2, W[:, 768:896], g0[:], start=False, stop=False)
            _mm(nc, p2, W[:, 896:1024], g1[:], start=False, stop=True)
            h3t = hp.tile([128, NT], F32R, tag="h")
            nc.scalar.activation(out=h3t[:], in_=p2[:], func=AF.Identity,
                                 bias=Bb[:, 6:7], scale=1.0)
            return h3t

        def emit_head(it, h):
            b0 = it * N
            yf = layernorm(h, tag="f")
            p3 = psB.tile([128, N], F32, tag="lnh")
            nc.tensor.matmul(p3[:], wh_t[:, 0:128], yf[:, 0:N],
                             start=True, stop=False)
            nc.tensor.matmul(p3[:], wh_t[:, 0:128], yf[:, N:NT],
                             start=False, stop=True)
            p1h = sp.tile([128, N], F32R, tag="p1h")
            nc.scalar.activation(out=p1h[:], in_=p3[:], func=AF.Gelu,
                                 bias=bh_t[:, 0:1], scale=1.0)
            p4 = psB.tile([64, N], F32, tag="lnh")
            nc.tensor.matmul(p4[:], wh_t[:, 128:192], p1h[:],
                             start=True, stop=True)
            p2h = sp.tile([64, N], F32R, tag="p2h")
            nc.scalar.activation(out=p2h[:], in_=p4[:], func=AF.Gelu,
                                 bias=bh_t[0:64, 1:2], scale=1.0)
            p5 = psB.tile([1, N], F32, tag="lnh")
            nc.tensor.matmul(p5[:], wh_t[0:64, 192:193], p2h[:],
                             start=True, stop=True)
            th = sp.tile([1, N], F32, tag="th")
            nc.scalar.activation(out=th[:], in_=p5[:], func=AF.Tanh,
                                 bias=bh_t[0:1, 2:3], scale=1.0)
            res = sp.tile([1, N], F32, tag="res")
            nc.gpsimd.tensor_scalar(res[:], th[:], 3.0, None, op0=ALU.mult)
            nc.sync.dma_start(o_d[0:1, b0:b0 + N], res[:])

        # software-pipeline pairs of super-tiles: interleave emission so the
        # scheduler fills one tile's serial-chain stalls with the other's work
        pairs = ntiles // 2
        for p in range(pairs):
            ia, ib = 2 * p, 2 * p + 1
            ha = emit_embed(ia)
            hb = emit_embed(ib)
            for l in range(L):
                ha = emit_layer(l, ha)
                hb = emit_layer(l, hb)
            emit_head(ia, ha)
            emit_head(ib, hb)
        for it in range(pairs * 2, ntiles):
            h = emit_embed(it)
            for l in range(L):
                h = emit_layer(l, h)
            emit_head(it, h)

    nc.compile()
    return nc


_NC_CACHE = {}


def kernel(**inputs):
    w = _prepare_weights(inputs)
    if "nc" not in _NC_CACHE:
        _NC_CACHE["nc"] = build_nc()
    nc = _NC_CACHE["nc"]
    x = np.asarray(inputs["x"], np.float32)
    in_maps = []
    for c in range(NCORES):
        in_maps.append({
            "x": np.ascontiguousarray(x[c * BP:(c + 1) * BP]),
            "wl": w["wl"], "bl": w["bl"], "w_inT": w["w_inT"],
            "eb": w["eb"], "wh": w["wh"], "bh": w["bh"],
        })
    res = run_bass_kernel_spmd(nc, in_maps, core_ids=list(range(NCORES)))
    outs = [res.results[c]["o"].reshape(BP, 1) for c in range(NCORES)]
    return np.concatenate(outs, axis=0).astype(np.float32)


if __name__ == "__main__":
    build_nc(ntiles=1)
    print("build ok")



# revision 22
# speedup vs baseline: 1.3148x; 1.3148x over previous
"""Trainium2 Bass kernel for nn_BCTransformer: B=131072 batch of tiny 2-token
4-layer transformer encoder forward passes.

Pure data parallel over 8 NeuronCores (16384 batch each).  Feature-major
layout: [D=128 partitions, columns], columns = (token, batch); super-tiles of
1024 columns (512 batch x 2 tokens); 32 super-tiles per core.

Centered-residual-stream design:
 - The residual state kept is hc = C@h (C = I - J/128, idempotent).  C is
   folded host-side into every weight that writes the stream, so LayerNorm
   needs NO centering matmul: var comes straight from Square(hc+cb) + an
   all-1/256 matmul.
 - hc lives in PSUM for the whole tile; residual adds are matmul
   accumulations (start=False) into the same banks.  Additive biases never
   enter hc: they are applied at each LN read point, riding the ACT Square
   bias and a fused (hc+cb)*R scalar_tensor_tensor.
 - rstd: magic-constant seed (DVE shift/xor + Pool int add) + one custom DVE
   op doing 2 Newton iterations with eps folded in via its C1 constant.
 - attention: softmax over S=2 == 0.5+0.5*tanh(d/2).  dk/dv are computed as
   W@(y0-y1) (k/v biases cancel); the (v0+v1) path is folded host-side into
   0.5*C@out_w@Wv so k/v are never materialized.  Score path runs in bf16.
"""
import sys

sys.path.insert(0, "/opt/trn_rl_repo")

import math
from contextlib import ExitStack

import numpy as np

import concourse.bass as bass
import concourse.tile as tile
from concourse import bacc, mybir
from concourse.bass_utils import run_bass_kernel_spmd

# ---------------------------------------------------------------- constants
D = 128
NH = 4
HD = 32
FF = 256
L = 4
S = 2
B = 131072
EPS = 1e-5
NCORES = 8
BP = B // NCORES          # batch per core = 16384
N = 256                   # batch elems per super-tile
NT = 2 * N                # columns per super-tile (tok0 | tok1) = 1 PSUM bank
NTILES = BP // N          # 64
ILV = 4                   # super-tiles in flight

F32 = mybir.dt.float32
F32R = mybir.dt.float32r
BF16 = mybir.dt.bfloat16
I32 = mybir.dt.int32
AF = mybir.ActivationFunctionType
ALU = mybir.AluOpType

# ------------------------------------------------- custom DVE rsqrt op
MAGIC = 0x5F375A86
MAGIC_VH = MAGIC - (1 << 22)   # seed computed from bits of vh = v/2
SEED_ADD = MAGIC_VH + 1        # seed = ~(i_vh >> 1) + SEED_ADD


def _register_rsqrt_op():
    """y = NR2(seed, vh) ~= 1/sqrt(2*vh); C0=1.5."""
    import concourse.dve_ops as dve_ops
    from concourse.dve_ops import DveOp
    from concourse.dve_spec import C0, Spec, Src0, Src1, lower, _has_src1
    from concourse.dve_uop import DveOpSpec

    name = "RSQRT_NR2_ANT"
    if name in dve_ops._SUB_OPCODE_FOR_NAME:
        for op in dve_ops.OPS:
            if op.name == name:
                return op

    def _ref(in0, in1, c0, c1, c2):
        vh = in0.astype(np.float32)
        s = in1.astype(np.float32)
        y1 = s * (c0 - vh * s * s)
        y2 = y1 * (c0 - vh * y1 * y1)
        return y2.astype(np.float32)

    _y1 = Src1 * (C0 - Src0 * (Src1 * Src1))
    spec = Spec(body=_y1 * (C0 - Src0 * (_y1 * _y1)), reference=_ref)
    opcode = dve_ops._CUSTOM_DVE_ROW_BASE + len(dve_ops.OPS)
    assert opcode < 0x20
    dve_ops._SUB_OPCODE_FOR_NAME[name] = opcode
    shas = {}
    for ver in ("v3", "v4"):
        try:
            uops = lower(spec, ver=ver)
            shas[ver] = DveOpSpec(
                name=name, opcode=opcode, uops=uops, rd1_en=_has_src1(spec)
            ).sha(ver)
        except Exception:
            pass
    op = DveOp(name, spec, subdim=False, uops_sha=shas)
    dve_ops.OPS.append(op)
    dve_ops.CUSTOM_DVE_SPECS[name] = spec
    return op


RSQRT_NR2 = _register_rsqrt_op()


# ------------------------------------------------- host-side weight folding
def _prepare_weights(p):
    f = lambda a: np.asarray(a, np.float64)
    C = np.eye(D) - 1.0 / D                      # centering projector

    out = {}
    # embed: ec = (C @ W_in sqrt(D)) @ t ; read-bias cbin = C @ (b_in sqrt(D))
    w_in = f(p["w_in"]) * math.sqrt(D)
    out["wx"] = np.ascontiguousarray((C @ w_in).T).astype(np.float32)  # [2,128]
    cbin = C @ (f(p["b_in"]) * math.sqrt(D))

    # positional encoding
    pos = np.arange(10, dtype=np.float64)[:, None]
    div = np.exp(np.arange(0, D, 2, dtype=np.float64) * (-math.log(10000.0) / D))
    pe = np.zeros((10, D))
    pe[:, 0::2] = np.sin(pos * div)
    pe[:, 1::2] = np.cos(pos * div)

    # hc0 = Cg @ y_e  (+ bias cpe_tok at later reads)
    Cg = C @ np.diag(f(p["g_in"]))
    out["wcg"] = np.ascontiguousarray(Cg.T).astype(np.float32)  # [128,128]
    bias0 = C @ (f(p["bt_in"]) + pe[0])
    bias1 = C @ (f(p["bt_in"]) + pe[1])

    ln_bias = [(cbin.copy(), cbin.copy())]       # embed-LN read point
    q_bias, g_bias = [], []
    wl = np.zeros((L, 128, 9 * 128), np.float32)
    for l in range(L):
        ln_bias.append((bias0.copy(), bias1.copy()))   # LN1 of layer l
        g1 = f(p["n1_g"][l]); b1 = f(p["n1_b"][l])
        qkv_w = f(p["qkv_w"][l]); qkv_b = f(p["qkv_b"][l])
        Wq, Wk, Wv = qkv_w[0:128], qkv_w[128:256], qkv_w[256:384]
        Wqg, Wkg, Wvg = Wq * g1, Wk * g1, Wv * g1
        bq = qkv_b[0:128] + Wq @ b1
        bv = qkv_b[256:384] + Wv @ b1
        out_w = f(p["out_w"][l])
        CWV = 0.5 * (C @ out_w @ Wvg)            # sy path
        CWh = 0.5 * (C @ out_w)                  # u path
        g2 = f(p["n2_g"][l]); b2 = f(p["n2_b"][l])
        ff1_w = f(p["ff1_w"][l]); ff1_b = f(p["ff1_b"][l])
        F1g = ff1_w * g2
        bf = ff1_b + ff1_w @ b2
        Mf2 = C @ f(p["ff2_w"][l])               # [128, 256]

        wl[l, :, 0 * 128:1 * 128] = Wqg.T
        wl[l, :, 1 * 128:2 * 128] = Wkg.T
        wl[l, :, 2 * 128:3 * 128] = Wvg.T
        wl[l, :, 3 * 128:4 * 128] = CWV.T
        wl[l, :, 4 * 128:5 * 128] = CWh.T
        wl[l, :, 5 * 128:6 * 128] = F1g[0:128].T
        wl[l, :, 6 * 128:7 * 128] = F1g[128:256].T
        wl[l, :, 7 * 128:8 * 128] = Mf2[:, 0:128].T
        wl[l, :, 8 * 128:9 * 128] = Mf2[:, 128:256].T
        q_bias.append(bq)
        g_bias.append((bf[0:128], bf[128:256]))

        # residual-stream bias accumulation (never added to hc itself)
        delta = C @ (f(p["out_b"][l]) + out_w @ bv)
        bias0 += delta; bias1 += delta
        ln_bias.append((bias0.copy(), bias1.copy()))   # LN2 of layer l
        delta2 = C @ f(p["ff2_b"][l])
        bias0 += delta2; bias1 += delta2
    ln_bias.append((bias0.copy(), bias1.copy()))       # final-LN read point
    out["wl"] = wl

    # head: p = 0.5*(y0+y1); h1 folds g_out and the 0.5
    go = f(p["g_out"]); bo = f(p["bt_out"])
    h1_w = f(p["h1_w"])
    wh = np.zeros((128, 193), np.float32)
    wh[:, 0:128] = (0.5 * h1_w * go).T
    wh[:, 128:192] = f(p["h2_w"]).T
    wh[0:64, 192] = f(p["h3_w"])[0]
    out["wh"] = wh
    bh1 = f(p["h1_b"]) + h1_w @ bo

    # bias tile: q (4), gelu (8), head (3)
    nb = 22 + 4 + 8 + 3
    bias = np.zeros((128, nb), np.float64)
    for l in range(L):
        bias[:, 22 + l] = q_bias[l]
        bias[:, 26 + 2 * l] = g_bias[l][0]
        bias[:, 27 + 2 * l] = g_bias[l][1]
    bias[:, 34] = bh1
    bias[0:64, 35] = f(p["h2_b"])
    bias[0, 36] = f(p["h3_b"])[0]
    out["bias"] = bias.astype(np.float32)
    # bias DELTAS accumulated straight into the psum residual stream
    # (all C-projected => mean-zero => the centered invariant holds):
    # row 0 cbin (into ec), 1 cpe0, 2 cpe1 (into hc init),
    # 3+2l attn delta, 4+2l ff delta
    bd = np.zeros((11, 128), np.float64)
    bd[0] = ln_bias[0][0]
    bd[1] = ln_bias[1][0]
    bd[2] = ln_bias[1][1]
    for l in range(L):
        bd[3 + 2 * l] = ln_bias[2 + 2 * l][0] - ln_bias[1 + 2 * l][0]
        if l < L - 1:
            bd[4 + 2 * l] = ln_bias[3 + 2 * l][0] - ln_bias[2 + 2 * l][0]
        else:
            bd[4 + 2 * l] = ln_bias[9][0] - ln_bias[2 + 2 * l][0]
    out["biasd"] = bd.astype(np.float32)
    return out


def _static_consts():
    c = {}
    c["Jv"] = np.full((128, 128), 1.0 / 256.0, np.float32)
    sm = np.zeros((128, 4), np.float32)
    for d in range(128):
        sm[d, d // HD] = 1.0 / math.sqrt(HD)
    c["smask"] = sm
    bc = np.zeros((36, 256), np.float32)
    for d in range(128):
        bc[0 + d // HD, 0 * 128 + d] = 1.0
        bc[32 + d // HD, 1 * 128 + d] = 1.0
    c["bcmask"] = bc
    return c


def r32(ap):
    return ap.bitcast(F32R)


def build_nc(ntiles=NTILES):
    nc = bacc.Bacc(None, target_bir_lowering=False)
    cst = _static_consts()

    x_d = nc.dram_tensor("x", [BP, 4], F32, kind="ExternalInput")
    wx_d = nc.dram_tensor("wx", [2, 128], F32, kind="ExternalInput")
    wcg_d = nc.dram_tensor("wcg", [128, 128], F32, kind="ExternalInput")
    wl_d = nc.dram_tensor("wl", [L, 128, 9 * 128], F32, kind="ExternalInput")
    wh_d = nc.dram_tensor("wh", [128, 193], F32, kind="ExternalInput")
    bias_d = nc.dram_tensor("bias", [128, 37], F32, kind="ExternalInput")
    biasd_d = nc.dram_tensor("biasd", [11, 128], F32, kind="ExternalInput")
    o_d = nc.dram_tensor("o", [1, BP], F32, kind="ExternalOutput")

    Jv_d = nc.inline_tensor(cst["Jv"], name="Jvmat")
    sm_d = nc.inline_tensor(cst["smask"], name="smask")
    bc_d = nc.inline_tensor(cst["bcmask"], name="bcmask")

    with tile.TileContext(nc) as tc, ExitStack() as ctx:
        wp = ctx.enter_context(tc.tile_pool(name="weights", bufs=1))
        sp = ctx.enter_context(tc.tile_pool(name="scratch", bufs=ILV))
        yp = ctx.enter_context(tc.tile_pool(name="ybuf", bufs=ILV))
        hcp = ctx.enter_context(tc.tile_pool(name="hc", bufs=ILV, space="PSUM"))
        ptr = ctx.enter_context(tc.tile_pool(name="ptrans", bufs=ILV,
                                             space="PSUM"))

        def wtile(src, shape, tag, dt=F32):
            if dt == F32:
                t = wp.tile(shape, F32, tag=tag)
                nc.sync.dma_start(t[:], src)
                return t
            st = sp.tile([128, 9 * 128], F32, tag="wstage")
            sv = st[: shape[0], : shape[1]]
            nc.sync.dma_start(sv, src)
            t = wp.tile(shape, dt, tag=tag)
            nc.scalar.copy(t[:], sv)
            return t

        wx_t = wtile(wx_d[:], [2, 128], "wx", F32R)
        wcg_t = wtile(wcg_d[:], [128, 128], "wcg", F32R)
        wl_t = [wtile(wl_d[l], [128, 9 * 128], f"wl{l}", F32R) for l in range(L)]
        wh_t = wtile(wh_d[:], [128, 193], "wh", F32R)
        bias_t = wtile(bias_d[:], [128, 37], "bias", F32)
        Jv_t = wtile(Jv_d[:], [128, 128], "Jv", F32R)
        sm_f = wtile(sm_d[:], [128, 4], "smf", F32)
        sm_t = wp.tile([128, 4], BF16, tag="smb")
        nc.vector.tensor_copy(sm_t[:], sm_f[:])
        bc_f = wtile(bc_d[:], [36, 256], "bcf", F32)
        bc_t = wp.tile([36, 256], BF16, tag="bcb")
        nc.vector.tensor_copy(bc_t[:], bc_f[:])

        bd_t = []
        for r in range(11):
            st = sp.tile([128, 9 * 128], F32, tag="wstage")
            nc.sync.dma_start(st[0:1, 0:128], biasd_d[r:r + 1, :])
            t = wp.tile([1, 128], F32R, tag=f"bd{r}")
            nc.scalar.copy(t[:], st[0:1, 0:128])
            bd_t.append(t)

        eps_st = sp.tile([128, 9 * 128], F32, tag="wstage")
        nc.vector.memset(eps_st[0:1, 0:128], EPS / 2.0)
        eps_t = wp.tile([1, 128], F32R, tag="epsr")
        nc.scalar.copy(eps_t[:], eps_st[0:1, 0:128])
        one_st = sp.tile([128, 9 * 128], F32, tag="wstage")
        nc.vector.memset(one_st[0:1, 0:NT], 1.0)
        ones_t = wp.tile([1, NT], F32R, tag="ones")
        nc.scalar.copy(ones_t[:], one_st[0:1, 0:NT])

        def bcol(i):
            return bias_t[:, i:i + 1]

        def layernorm(hc_ps, vh, sc, eps=False):
            """y = hc * rstd (bias already accumulated inside hc)."""
            sq, tb, R, y = sc
            nc.scalar.activation(out=sq[:], in_=hc_ps[:],
                                 func=AF.Square, bias=0.0, scale=1.0)
            if eps:
                nc.tensor.matmul(vh[:], eps_t[:], ones_t[:],
                                 start=True, stop=False)
            nc.tensor.matmul(vh[:], Jv_t[:], sq[:], start=not eps, stop=True)
            nc.vector.tensor_scalar(
                tb[:], vh[:].bitcast(I32), 1, -1,
                op0=ALU.logical_shift_right, op1=ALU.bitwise_xor)
            nc.gpsimd.tensor_scalar(tb[:], tb[:], SEED_ADD, None, op0=ALU.add)
            nc.vector._custom_dve(RSQRT_NR2, out=R[:], in0=vh[:],
                                  in1=tb[:].bitcast(F32), s0=1.5)
            nc.vector.tensor_mul(y[:], hc_ps[:], R[:])
            return y

        def emit_embed(it):
            b0 = it * N
            xs = sp.tile([2, 2, N], F32, tag="xs")
            nc.sync.dma_start(
                xs[:, 0, :], x_d[b0:b0 + N, 0:2].rearrange("n f -> f n"))
            nc.sync.dma_start(
                xs[:, 1, :], x_d[b0:b0 + N, 2:4].rearrange("n f -> f n"))
            ta = ptr.tile([128, NT], F32, tag="big")
            hc = hcp.tile([128, NT], F32, tag="hc")
            sq_t = sp.tile([128, NT], F32R, tag="sq")
            tb_t = sp.tile([128, NT], I32, tag="tbits")
            r_t = sp.tile([128, NT], F32, tag="rstd")
            y_t = yp.tile([128, NT], F32R, tag="yln")
            sc = (sq_t, tb_t, r_t, y_t)
            xr = sp.tile([2, 2, N], F32R, tag="xr")
            nc.gpsimd.tensor_copy(xr[:], xs[:])
            ec = ta
            nc.tensor.matmul(ec[:, 0:N], wx_t[:], xr[:, 0, :],
                             start=True, stop=False)
            nc.tensor.matmul(ec[:, N:NT], wx_t[:], xr[:, 1, :],
                             start=False, stop=False)
            nc.tensor.matmul(ec[:], bd_t[0][:], ones_t[:],
                             start=False, stop=True)
            # embed-LN variance scratch borrows the hc bank: hc is only
            # written (start=True, clearing it) after y_e is complete.
            y_e = layernorm(ec, hc, sc, eps=True)
            nc.tensor.matmul(hc[:], wcg_t[:], y_e[:], start=True, stop=False)
            nc.tensor.matmul(hc[:, 0:N], bd_t[1][:], ones_t[:, 0:N],
                             start=False, stop=False)
            nc.tensor.matmul(hc[:, N:NT], bd_t[2][:], ones_t[:, N:NT],
                             start=False, stop=True)
            return hc, ta, sc

        def emit_layer(l, hs):
            hc, ta, sc = hs
            W = wl_t[l]
            Wq, Wk, Wv = W[:, 0:128], W[:, 128:256], W[:, 256:384]
            CWV, CWh = W[:, 384:512], W[:, 512:640]
            F0, F1 = W[:, 640:768], W[:, 768:896]
            M2a, M2b = W[:, 896:1024], W[:, 1024:1152]

            y1 = layernorm(hc, ta, sc)
            y1f = y1[:].bitcast(F32)
            dy = yp.tile([128, N], F32R, tag="dy")
            nc.gpsimd.tensor_tensor(dy[:], y1f[:, 0:N], y1f[:, N:NT],
                                    op=ALU.subtract)
            sy = yp.tile([128, N], F32R, tag="sy")
            nc.gpsimd.tensor_tensor(sy[:], y1f[:, 0:N], y1f[:, N:NT],
                                    op=ALU.add)
            q_ps = ta
            nc.tensor.matmul(q_ps[:], Wq, y1[:], start=True, stop=True)
            q_sb = sp.tile([128, NT], BF16, tag="qsb")
            nc.scalar.activation(out=q_sb[:], in_=q_ps[:], func=AF.Identity,
                                 bias=bcol(22 + l), scale=1.0)
            kv_ps = ta
            nc.tensor.matmul(kv_ps[:, 0:N], Wk, dy[:], start=True, stop=True)
            nc.tensor.matmul(kv_ps[:, N:NT], Wv, dy[:], start=True, stop=True)
            kv_sb = sp.tile([128, NT], BF16, tag="kvsb")
            nc.scalar.activation(out=kv_sb[:], in_=kv_ps[:], func=AF.Identity,
                                 bias=0.0, scale=1.0)
            dk, dv = kv_sb[:, 0:N], kv_sb[:, N:NT]
            pr = sp.tile([128, 2, N], BF16, tag="prods")
            dk_b = bass.AP(tensor=dk.tensor, offset=dk.offset,
                           ap=[dk.ap[0], [0, 2], dk.ap[1]])
            nc.gpsimd.tensor_mul(
                pr[:], q_sb[:].rearrange("p (q n) -> p q n", q=2), dk_b)
            at = ta
            nc.tensor.matmul(at[0:4, 0:N], sm_t[:], pr[:, 0, :],
                             start=True, stop=True)
            nc.tensor.matmul(at[32:36, 0:N], sm_t[:], pr[:, 1, :],
                             start=True, stop=True, tile_position=(0, 32))
            T8 = sp.tile([36, N], BF16, tag="T8")
            nc.scalar.activation(out=T8[:], in_=at[0:36, 0:N],
                                 func=AF.Tanh, bias=0.0, scale=0.5)
            nc.tensor.matmul(at[:, 0:N], bc_t[:, 0:128], T8[:],
                             start=True, stop=True)
            nc.tensor.matmul(at[:, N:NT], bc_t[:, 128:256], T8[:],
                             start=True, stop=True)
            u = yp.tile([128, NT], F32R, tag="u")
            dv_b = bass.AP(tensor=dv.tensor, offset=dv.offset,
                           ap=[dv.ap[0], [0, 2], dv.ap[1]])
            nc.vector.tensor_mul(
                u[:].rearrange("p (q n) -> p q n", q=2),
                at[:].rearrange("p (q n) -> p q n", q=2), dv_b)
            nc.tensor.matmul(hc[:, 0:N], CWV, sy[:], start=False, stop=False,
                             skip_group_check=True)
            nc.tensor.matmul(hc[:, N:NT], CWV, sy[:], start=False, stop=False,
                             skip_group_check=True)
            nc.tensor.matmul(hc[:], CWh, u[:], start=False, stop=False,
                             skip_group_check=True)
            nc.tensor.matmul(hc[:], bd_t[3 + 2 * l][:], ones_t[:],
                             start=False, stop=True, skip_group_check=True)

            y2 = layernorm(hc, ta, sc)
            f0 = ta
            nc.tensor.matmul(f0[:], F0, y2[:], start=True, stop=True)
            g0 = yp.tile([128, NT], F32R, tag="g0")
            nc.scalar.activation(out=g0[:], in_=f0[:], func=AF.Gelu,
                                 bias=bcol(26 + 2 * l), scale=1.0)
            f1 = ta
            nc.tensor.matmul(f1[:], F1, y2[:], start=True, stop=True)
            g1 = yp.tile([128, NT], F32R, tag="g1")
            nc.scalar.activation(out=g1[:], in_=f1[:], func=AF.Gelu,
                                 bias=bcol(27 + 2 * l), scale=1.0)
            nc.tensor.matmul(hc[:], M2a, g0[:], start=False, stop=False,
                             skip_group_check=True)
            nc.tensor.matmul(hc[:], M2b, g1[:], start=False, stop=False,
                             skip_group_check=True)
            nc.tensor.matmul(hc[:], bd_t[4 + 2 * l][:], ones_t[:],
                             start=False, stop=True, skip_group_check=True)
            return hc, ta, sc

        def emit_head(it, hs):
            hc, ta, sc = hs
            b0 = it * N
            yf = layernorm(hc, ta, sc)
            pp = ta
            p3 = pp[:, 0:N]
            nc.tensor.matmul(p3, wh_t[:, 0:128], yf[:, 0:N],
                             start=True, stop=False)
            nc.tensor.matmul(p3, wh_t[:, 0:128], yf[:, N:NT],
                             start=False, stop=True)
            p1h = sp.tile([128, N], F32R, tag="p1h")
            nc.scalar.activation(out=p1h[:], in_=p3, func=AF.Gelu,
                                 bias=bcol(34), scale=1.0)
            p4 = pp[0:64, N:NT]
            nc.tensor.matmul(p4, wh_t[:, 128:192], p1h[:],
                             start=True, stop=True)
            p2h = sp.tile([64, N], F32R, tag="p2h")
            nc.scalar.activation(out=p2h[:], in_=p4, func=AF.Gelu,
                                 bias=bias_t[0:64, 35:36], scale=1.0)
            p5 = pp[0:1, 0:N]
            nc.tensor.matmul(p5, wh_t[0:64, 192:193], p2h[:],
                             start=True, stop=True)
            th = sp.tile([1, N], F32, tag="th")
            nc.scalar.activation(out=th[:], in_=p5, func=AF.Tanh,
                                 bias=bias_t[0:1, 36:37], scale=1.0)
            res = sp.tile([1, N], F32, tag="res")
            nc.gpsimd.tensor_scalar(res[:], th[:], 3.0, None, op0=ALU.mult)
            nc.sync.dma_start(o_d[0:1, b0:b0 + N], res[:])

        it0 = 0
        while it0 < ntiles:
            g = min(ILV, ntiles - it0)
            hs = [emit_embed(it0 + j) for j in range(g)]
            for l in range(L):
                hs = [emit_layer(l, h) for h in hs]
            for j in range(g):
                emit_head(it0 + j, hs[j])
            it0 += g

    nc.compile()
    return nc


_NC_CACHE = {}


def kernel(**inputs):
    w = _prepare_weights(inputs)
    if "nc" not in _NC_CACHE:
        _NC_CACHE["nc"] = build_nc()
    nc = _NC_CACHE["nc"]
    x = np.asarray(inputs["x"], np.float32)
    in_maps = []
    for c in range(NCORES):
        in_maps.append({
            "x": np.ascontiguousarray(x[c * BP:(c + 1) * BP]),
            "wx": w["wx"], "wcg": w["wcg"], "wl": w["wl"],
            "wh": w["wh"], "bias": w["bias"], "biasd": w["biasd"],
        })
    res = run_bass_kernel_spmd(nc, in_maps, core_ids=list(range(NCORES)))
    outs = [res.results[c]["o"].reshape(BP, 1) for c in range(NCORES)]
    return np.concatenate(outs, axis=0).astype(np.float32)


if __name__ == "__main__":
    build_nc(ntiles=2)
    print("build ok")
